# revision 16
# baseline (speedup 1.0000x reference)
"""Trainium2 Bass kernel for nn_EvoAttentionCausalTorch.

Reference math (per (b,h) slice, V: [L, D], D=128):
    ctx   = cumsum_l(V)
    cm    = ctx / t                      (t = 1..L)
    csg   = sigmoid(cm @ Wg + bg)
    s     = cumsum_l(csg * V)
    li    = V @ Wv + bv
    den   = |s| + |li| + 1e-8
    q     = V @ Wq + bq
    gate  = silu(q / den) * V
    fg    = sigmoid(gate @ Wf + bf)
    alive = (sum_d |V| > 0)
    y     = fg * ctx * alive
    out   = LN_d(y) * gamma + beta

Device mapping (per core, 8 of the 64 (b*h) slices):
  - "transposed" layout: d=128 on partitions, l on the free dim.
  - cumsums     -> DVE tensor_tensor_scan (prefix scan along free dim)
  - linears     -> PE matmul, lhsT = W (d_in x d_out), rhs = x^T chunks
  - sigmoid/abs/square -> ACT (single 'sigmoid_and_others' table set);
    silu(x) = x*sigmoid(x) with the multiply on DVE.
  - division    -> DVE reciprocal
  - LN stats    -> PE ones-matmul reductions (M=32 replicated col-tiles),
                   evicted via ACT, reshaped to [128, L/128] via SBUF DMA,
                   rsqrt = ACT Sqrt + DVE reciprocal + 1 Newton step.
  - LN apply    -> per-l rows A=rstd*alive, B=-mu*A broadcast via K=1
                   matmuls with lhsT=gamma row: E=gamma*A, Dp=gamma*B;
                   out = y*E + Dp + beta.
"""

import os
import sys
from contextlib import ExitStack

import numpy as np

for _p in ("/opt/trn_rl_repo", "/root/.axon_site/_ro/trn_rl_repo"):
    if os.path.isdir(_p) and _p not in sys.path:
        sys.path.insert(0, _p)

import ml_dtypes  # noqa: E402
import concourse.bass as bass  # noqa: E402
import concourse.mybir as mybir  # noqa: E402
import concourse.tile as tile  # noqa: E402

from concourse.vector_clock import ScopedClock  # noqa: E402

AF = mybir.ActivationFunctionType
OP = mybir.AluOpType
F32 = mybir.dt.float32
BF16 = mybir.dt.bfloat16

class SplitDrainTileContext(tile.TileContext):
    """TileContext that never leaves more than one semaphore wait on any
    instruction. The walrus build in this environment rejects instructions
    with multiple sync-wait commands ("Too many sync wait commands"), while
    the Tile scheduler freely emits them. Excess waits are hoisted onto
    same-engine NOP instructions inserted immediately before the owner."""

    def _split_multiwait_insts(self):
        nc = self.nc
        cur_bb = nc.cur_bb.bb
        for blk in nc.m.functions[0].blocks:
            insts = list(blk.instructions)
            if not any(
                i.sync_info is not None and len(i.sync_info.on_wait) > 1
                for i in insts
            ):
                continue
            out = []
            for inst in insts:
                si = inst.sync_info
                if si is not None and len(si.on_wait) > 1:
                    waits = list(si.on_wait)
                    eng = nc.engines[inst.engine]
                    for w in waits[:-1]:
                        nop = eng.nop(nofuse=True, hint="wait_split").ins
                        # nop() appended to the current bb; relocate it.
                        cl = cur_bb.instructions
                        assert cl[-1] is nop
                        cl.pop()
                        nop.sync_info = mybir.SyncInfo(
                            on_wait=[w], on_update=[])
                        out.append(nop)
                    inst.sync_info = mybir.SyncInfo(
                        on_wait=[waits[-1]], on_update=list(si.on_update))
                out.append(inst)
            blk.instructions.clear()
            blk.instructions.extend(out)

    def _drain_and_barrier(self, tick_clock, wait_clock):
        nc = self.nc
        self._split_multiwait_insts()
        drain_inst = nc.sync.drain()
        wait_clock.add_sem_waits(
            drain_inst.ins, ScopedClock({None: tick_clock.global_clock})
        )
        si = drain_inst.ins.sync_info
        if si is not None and len(si.on_wait) > 1:
            waits = list(si.on_wait)
            drain_inst.ins.sync_info = mybir.SyncInfo(
                on_wait=[waits[0]], on_update=list(si.on_update))
            for w in waits[1:]:
                d2 = nc.sync.drain()
                d2.ins.sync_info = mybir.SyncInfo(on_wait=[w], on_update=[])

        nc.all_engine_barrier()
        assert self.sems is not None
        popped = nc._tile_sem_poison_stack.pop()
        assert popped is self._sem_poison
        nc.clear_and_free_semaphores(list(self.sems.allocated().values()))
        nc.all_engine_barrier()


B, H, L, D = 4, 16, 4096, 128
NCORES = 8
S = (B * H) // NCORES  # slices per core
CH = 512               # chunk (free-dim) size; one PSUM bank of fp32
LN_EPS = 1e-5


def build_nc(S=S, L=L, CH=CH):
    NCH = L // CH
    Q = CH // 128  # sub-index within a chunk: l = CH*c + Q*p + q

    nc = bass.Bass(trn_type="TRN2")

    vt_d = nc.declare_dram_parameter("vt", [S, 128, L], F32, isOutput=False)
    wg_d = nc.declare_dram_parameter("wg", [128, 128], F32, isOutput=False)
    wv_d = nc.declare_dram_parameter("wv", [128, 128], F32, isOutput=False)
    wq_d = nc.declare_dram_parameter("wq", [128, 128], F32, isOutput=False)
    wf_d = nc.declare_dram_parameter("wf", [128, 128], F32, isOutput=False)
    bg_d = nc.declare_dram_parameter("bg", [128, 1], F32, isOutput=False)
    bv_d = nc.declare_dram_parameter("bv", [128, 1], F32, isOutput=False)
    bq_d = nc.declare_dram_parameter("bq", [128, 1], F32, isOutput=False)
    bf_d = nc.declare_dram_parameter("bf", [128, 1], F32, isOutput=False)
    grow_d = nc.declare_dram_parameter("grow", [1, 128], F32, isOutput=False)
    beta_d = nc.declare_dram_parameter("beta", [128, 1], F32, isOutput=False)
    invt_d = nc.declare_dram_parameter("invt", [128, L], F32, isOutput=False)
    out_d = nc.declare_dram_parameter("out_t", [S, 128, L], F32, isOutput=True)

    with SplitDrainTileContext(nc) as tc:
        with ExitStack() as ctx:
            const = ctx.enter_context(tc.tile_pool(name="const", bufs=1))
            big = ctx.enter_context(tc.tile_pool(name="big", bufs=2))
            mid = ctx.enter_context(tc.tile_pool(name="mid", bufs=2))
            sm = ctx.enter_context(tc.tile_pool(name="sm", bufs=2))
            pmm = ctx.enter_context(tc.tile_pool(name="pmm", bufs=3, space="PSUM"))
            pst = ctx.enter_context(tc.tile_pool(name="pst", bufs=2, space="PSUM"))
            ped = ctx.enter_context(tc.tile_pool(name="ped", bufs=1, space="PSUM"))

            # ---- constants ----
            def cload(name, shape, dt, dram):
                t = const.tile(shape, dt, tag=name)
                nc.sync.dma_start(out=t, in_=dram[:, :])
                return t

            wg = cload("wg", [128, 128], F32, wg_d)
            wv = cload("wv", [128, 128], F32, wv_d)
            wq = cload("wq", [128, 128], F32, wq_d)
            wf = cload("wf", [128, 128], F32, wf_d)
            bg_c = cload("bg", [128, 1], F32, bg_d)
            bv_c = cload("bv", [128, 1], F32, bv_d)
            bq_c = cload("bq", [128, 1], F32, bq_d)
            bf_c = cload("bf", [128, 1], F32, bf_d)
            grow = cload("grow", [1, 128], F32, grow_d)
            beta_c = cload("beta", [128, 1], F32, beta_d)
            invt = cload("invt", [128, L], F32, invt_d)
            ones_f = const.tile([128, 32], F32, tag="ones_f")
            nc.vector.memset(ones_f, 1.0)
            ones_b = const.tile([128, 32], BF16, tag="ones_b")
            nc.vector.memset(ones_b, 1.0)

            for s in range(S):
                vt = big.tile([128, L], F32, tag="vt")
                nc.sync.dma_start(out=vt, in_=vt_d[s, :, :])

                # ctx is kept as per-chunk LOCAL cumsums plus a per-partition
                # chunk offset (oc): true ctx[:, c*CH+j] = ctxt[:, c*CH+j] +
                # oc[c]. Bounding each scan to CH elements keeps fp32
                # accumulation noise ~sqrt(L/CH) lower; the offset add is
                # folded into downstream op slots for free.
                ctxt = big.tile([128, L], F32, tag="ctx")
                oc = [None] * (NCH + 1)
                oc0 = sm.tile([128, 1], F32, tag="oc0")
                nc.vector.memset(oc0, 0.0)
                oc[0] = oc0
                for c in range(NCH):
                    cs = slice(c * CH, (c + 1) * CH)
                    nc.vector.tensor_tensor_scan(
                        out=ctxt[:, cs], data0=vt[:, cs], data1=vt[:, cs],
                        initial=0.0, op0=OP.add, op1=OP.bypass)
                    if c + 1 <= NCH - 1:
                        oc_next = sm.tile([128, 1], F32, tag=f"oc{c + 1}")
                        nc.vector.tensor_scalar(
                            out=oc_next,
                            in0=ctxt[:, (c + 1) * CH - 1:(c + 1) * CH],
                            scalar1=oc[c], scalar2=None, op0=OP.add)
                        oc[c + 1] = oc_next

                y = big.tile([128, L], F32, tag="y")
                mu_t = sm.tile([128, NCH, Q], F32, tag="mu")
                sq_t = sm.tile([128, NCH, Q], F32, tag="sq")
                zz_t = sm.tile([128, NCH, Q], F32, tag="zz")

                os_ = [None] * (NCH + 1)
                os0 = sm.tile([128, 1], F32, tag="os0")
                nc.vector.memset(os0, 0.0)
                os_[0] = os0
                for c in range(NCH):
                    cs = slice(c * CH, (c + 1) * CH)

                    # causal-mean gate: csg = sigmoid(Wg@(ctx/t) + bg)
                    cm = mid.tile([128, CH], F32, tag="cm")
                    nc.vector.scalar_tensor_tensor(
                        out=cm, in0=ctxt[:, cs], scalar=oc[c],
                        in1=invt[:, cs], op0=OP.add, op1=OP.mult)
                    pg = pmm.tile([128, CH], F32, tag="mm")
                    nc.tensor.matmul(out=pg, lhsT=wg, rhs=cm, start=True, stop=True)
                    t1 = mid.tile([128, CH], F32, tag="t1")
                    nc.scalar.activation(out=t1, in_=pg, func=AF.Sigmoid,
                                         bias=bg_c, scale=1.0)
                    sv = mid.tile([128, CH], F32, tag="sv")
                    nc.vector.tensor_mul(out=sv, in0=t1, in1=vt[:, cs])
                    # s = cumsum(csg*V): local scan + per-partition offset
                    s2 = mid.tile([128, CH], F32, tag="s2")
                    nc.vector.tensor_tensor_scan(
                        out=s2, data0=sv, data1=sv, initial=0.0,
                        op0=OP.add, op1=OP.bypass)
                    if c + 1 <= NCH - 1:
                        os_next = sm.tile([128, 1], F32, tag=f"os{c + 1}")
                        nc.vector.tensor_scalar(
                            out=os_next, in0=s2[:, CH - 1:CH],
                            scalar1=os_[c], scalar2=None, op0=OP.add)
                        os_[c + 1] = os_next

                    # local_info: |mmv + bv|
                    pv = pmm.tile([128, CH], F32, tag="mm")
                    nc.tensor.matmul(out=pv, lhsT=wv, rhs=vt[:, cs],
                                     start=True, stop=True)
                    ali = mid.tile([128, CH], F32, tag="ali")
                    nc.scalar.activation(out=ali, in_=pv, func=AF.Abs,
                                         bias=bv_c, scale=1.0)
                    # den = (|s2 + os| + 1e-8) + |li| ; offset folds into Abs bias
                    as_ = mid.tile([128, CH], F32, tag="as")
                    nc.scalar.activation(out=as_, in_=s2, func=AF.Abs,
                                         bias=os_[c], scale=1.0)
                    den = mid.tile([128, CH], F32, tag="den")
                    nc.vector.scalar_tensor_tensor(
                        out=den, in0=as_, scalar=1e-8, in1=ali,
                        op0=OP.add, op1=OP.add)
                    rr = mid.tile([128, CH], F32, tag="rr")
                    nc.vector.reciprocal(out=rr, in_=den)

                    # qn = (mmq + bq)/den ; silu(qn) = qn*sigmoid(qn)
                    pq = pmm.tile([128, CH], F32, tag="mm")
                    nc.tensor.matmul(out=pq, lhsT=wq, rhs=vt[:, cs],
                                     start=True, stop=True)
                    qh = mid.tile([128, CH], F32, tag="qh")
                    nc.vector.scalar_tensor_tensor(
                        out=qh, in0=pq, scalar=bq_c, in1=rr,
                        op0=OP.add, op1=OP.mult)
                    s3 = mid.tile([128, CH], F32, tag="s3")
                    nc.scalar.activation(out=s3, in_=qh, func=AF.Sigmoid,
                                         bias=0.0, scale=1.0)
                    sl_ = mid.tile([128, CH], F32, tag="sl")
                    nc.vector.tensor_mul(out=sl_, in0=qh, in1=s3)
                    gate = mid.tile([128, CH], F32, tag="gate")
                    nc.vector.tensor_mul(out=gate, in0=sl_, in1=vt[:, cs])

                    # final gate: y = sigmoid(mmf + bf) * ctx
                    pf = pmm.tile([128, CH], F32, tag="mm")
                    nc.tensor.matmul(out=pf, lhsT=wf, rhs=gate,
                                     start=True, stop=True)
                    t2 = mid.tile([128, CH], F32, tag="t2")
                    nc.scalar.activation(out=t2, in_=pf, func=AF.Sigmoid,
                                         bias=bf_c, scale=1.0)
                    # y = fg * (ctx_local + oc)
                    nc.vector.scalar_tensor_tensor(
                        out=y[:, cs], in0=ctxt[:, cs], scalar=oc[c],
                        in1=t2, op0=OP.add, op1=OP.mult)

                    # LN stats + alive rows via PE reductions
                    y2 = mid.tile([128, CH], F32, tag="y2")
                    nc.scalar.activation(out=y2, in_=y[:, cs], func=AF.Square)
                    av = mid.tile([128, CH], BF16, tag="av")
                    nc.scalar.activation(out=av, in_=vt[:, cs], func=AF.Abs)

                    pstat = pst.tile([96, CH], F32, tag="st")
                    nc.tensor.matmul(out=pstat[0:32, :], lhsT=ones_f,
                                     rhs=y[:, cs], start=True, stop=True,
                                     tile_position=(0, 0))
                    nc.tensor.matmul(out=pstat[32:64, :], lhsT=ones_f,
                                     rhs=y2, start=True, stop=True,
                                     tile_position=(0, 32))
                    nc.tensor.matmul(out=pstat[64:96, :], lhsT=ones_b,
                                     rhs=av, start=True, stop=True,
                                     tile_position=(0, 64))
                    strows = mid.tile([96, CH], F32, tag="strows")
                    nc.scalar.activation(out=strows, in_=pstat, func=AF.Copy)
                    # reshape rows [1, CH] -> [128, Q] ; l = CH*c + Q*p + q
                    nc.sync.dma_start(out=mu_t[:, c, :], in_=strows[0:1, :])
                    nc.sync.dma_start(out=sq_t[:, c, :], in_=strows[32:33, :])
                    nc.sync.dma_start(out=zz_t[:, c, :], in_=strows[64:65, :])

                # ---- per-slice LN row math on [128, NCH*Q] ----
                mu_n = sm.tile([128, NCH, Q], F32, tag="mun")
                nc.vector.tensor_scalar(out=mu_n, in0=mu_t, scalar1=1.0 / 128,
                                        scalar2=None, op0=OP.mult)
                m2 = sm.tile([128, NCH, Q], F32, tag="m2")
                nc.vector.tensor_mul(out=m2, in0=mu_n, in1=mu_n)
                w2 = sm.tile([128, NCH, Q], F32, tag="w2")
                nc.vector.scalar_tensor_tensor(
                    out=w2, in0=sq_t, scalar=1.0 / 128, in1=m2,
                    op0=OP.mult, op1=OP.subtract)
                nc.vector.tensor_scalar(out=w2, in0=w2, scalar1=LN_EPS,
                                        scalar2=None, op0=OP.add)
                std0 = sm.tile([128, NCH, Q], F32, tag="std0")
                nc.scalar.activation(out=std0, in_=w2, func=AF.Sqrt)
                rs0 = sm.tile([128, NCH, Q], F32, tag="rs0")
                nc.vector.reciprocal(out=rs0, in_=std0)
                # one Newton refinement: rs1 = rs0*(1.5 - 0.5*w2*rs0^2)
                tn = sm.tile([128, NCH, Q], F32, tag="tn")
                nc.vector.tensor_mul(out=tn, in0=w2, in1=rs0)
                tn2 = sm.tile([128, NCH, Q], F32, tag="tn2")
                nc.vector.tensor_mul(out=tn2, in0=tn, in1=rs0)
                un = sm.tile([128, NCH, Q], F32, tag="un")
                nc.vector.tensor_scalar(out=un, in0=tn2, scalar1=-0.5,
                                        scalar2=1.5, op0=OP.mult, op1=OP.add)
                rs1 = sm.tile([128, NCH, Q], F32, tag="rs1")
                nc.vector.tensor_mul(out=rs1, in0=rs0, in1=un)
                # alive mask and final per-l factors
                af = sm.tile([128, NCH, Q], F32, tag="af")
                nc.vector.tensor_scalar(out=af, in0=zz_t, scalar1=0.0,
                                        scalar2=None, op0=OP.is_gt)
                A_t = sm.tile([128, NCH, Q], F32, tag="A")
                nc.vector.tensor_mul(out=A_t, in0=rs1, in1=af)
                B_t = sm.tile([128, NCH, Q], F32, tag="Bt")
                nc.vector.scalar_tensor_tensor(
                    out=B_t, in0=mu_n, scalar=-1.0, in1=A_t,
                    op0=OP.mult, op1=OP.mult)

                # ---- apply: out = y*(gamma.A) + gamma.B + beta ----
                for c in range(NCH):
                    cs = slice(c * CH, (c + 1) * CH)
                    ab = mid.tile([1, 2 * CH], F32, tag="ab")
                    nc.sync.dma_start(out=ab[0:1, 0:CH], in_=A_t[:, c, :])
                    nc.sync.dma_start(out=ab[0:1, CH:2 * CH], in_=B_t[:, c, :])
                    pe = ped.tile([128, 2 * CH], F32, tag="ed")
                    nc.tensor.matmul(out=pe[:, 0:CH], lhsT=grow,
                                     rhs=ab[0:1, 0:CH], start=True, stop=True)
                    nc.tensor.matmul(out=pe[:, CH:2 * CH], lhsT=grow,
                                     rhs=ab[0:1, CH:2 * CH], start=True, stop=True)
                    zc = mid.tile([128, CH], F32, tag="zc")
                    nc.vector.tensor_mul(out=zc, in0=y[:, cs], in1=pe[:, 0:CH])
                    ou = mid.tile([128, CH], F32, tag="ou")
                    nc.vector.scalar_tensor_tensor(
                        out=ou, in0=zc, scalar=beta_c, in1=pe[:, CH:2 * CH],
                        op0=OP.add, op1=OP.add)
                    nc.sync.dma_start(out=out_d[s, :, cs], in_=ou)
    return nc


def host_consts(Wg, bg, Wv, bv, Wq, bq, Wf, bf, gamma, beta, L=L):
    t = np.arange(1, L + 1, dtype=np.float64)
    invt = np.broadcast_to((1.0 / t).astype(np.float32), (128, L))
    return {
        "wg": np.ascontiguousarray(Wg, dtype=np.float32),
        "wv": np.ascontiguousarray(Wv, dtype=np.float32),
        "wq": np.ascontiguousarray(Wq, dtype=np.float32),
        "wf": np.ascontiguousarray(Wf, dtype=np.float32),
        "bg": np.asarray(bg, dtype=np.float32).reshape(128, 1),
        "bv": np.asarray(bv, dtype=np.float32).reshape(128, 1),
        "bq": np.asarray(bq, dtype=np.float32).reshape(128, 1),
        "bf": np.asarray(bf, dtype=np.float32).reshape(128, 1),
        "grow": np.asarray(gamma, dtype=np.float32).reshape(1, 128),
        "beta": np.asarray(beta, dtype=np.float32).reshape(128, 1),
        "invt": np.ascontiguousarray(invt),
    }


_NC_CACHE = {}


def _get_nc():
    key = (S, L, CH)
    if key not in _NC_CACHE:
        _NC_CACHE[key] = build_nc(*key)
    return _NC_CACHE[key]


def run_kernel(inputs, trace=False):
    """Returns (output [B,H,L,D] fp32, exec_time_ns or None)."""
    from concourse.bass_utils import run_bass_kernel_spmd

    V = np.asarray(inputs["V"], dtype=np.float32)
    consts = host_consts(
        np.asarray(inputs["Wg"]), np.asarray(inputs["bg"]),
        np.asarray(inputs["Wv"]), np.asarray(inputs["bv"]),
        np.asarray(inputs["Wq"]), np.asarray(inputs["bq"]),
        np.asarray(inputs["Wf"]), np.asarray(inputs["bf"]),
        np.asarray(inputs["gamma"]), np.asarray(inputs["beta"]),
    )
    Vr = V.reshape(B * H, L, D)
    in_maps = []
    for c in range(NCORES):
        sl = Vr[c * S:(c + 1) * S]                       # [S, L, D]
        vt = np.ascontiguousarray(sl.transpose(0, 2, 1))  # [S, D, L]
        m = {"vt": vt}
        m.update(consts)
        in_maps.append(m)

    nc = _get_nc()
    res = run_bass_kernel_spmd(nc, in_maps, list(range(NCORES)), trace=trace)
    outs = [res.results[c]["out_t"] for c in range(NCORES)]
    out = np.concatenate(outs, axis=0)                   # [B*H, D, L]
    out = out.transpose(0, 2, 1).reshape(B, H, L, D)
    return np.ascontiguousarray(out, dtype=np.float32), res.exec_time_ns


def kernel(**inputs):
    out, _ = run_kernel(inputs, trace=False)
    return out


def _in_maps_from_inputs(inputs):
    V = np.asarray(inputs["V"], dtype=np.float32)
    consts = host_consts(
        np.asarray(inputs["Wg"]), np.asarray(inputs["bg"]),
        np.asarray(inputs["Wv"]), np.asarray(inputs["bv"]),
        np.asarray(inputs["Wq"]), np.asarray(inputs["bq"]),
        np.asarray(inputs["Wf"]), np.asarray(inputs["bf"]),
        np.asarray(inputs["gamma"]), np.asarray(inputs["beta"]),
    )
    Vr = V.reshape(B * H, L, D)
    in_maps = []
    for c in range(NCORES):
        sl = Vr[c * S:(c + 1) * S]
        vt = np.ascontiguousarray(sl.transpose(0, 2, 1))
        m = {"vt": vt}
        m.update(consts)
        in_maps.append(m)
    return in_maps


def time_kernel(inputs, iters=12, reps=3):
    """Estimate per-invocation NEFF execution time by chaining `iters`
    back-to-back bass_exec calls inside one jitted program (the outputs of
    call i feed the donated output buffers of call i+1, forcing sequential
    execution and defeating CSE). Returns (ns_per_iter, details)."""
    import jax
    from jax.experimental.shard_map import shard_map
    from jax.sharding import Mesh, PartitionSpec
    import time as _time

    from concourse import bass2jax, mybir as mb
    from concourse.bass2jax import (
        _bass_exec_p, install_neuronx_cc_hook, partition_id_tensor,
    )

    install_neuronx_cc_hook()
    nc = _get_nc()
    in_maps = _in_maps_from_inputs(inputs)

    pid_name = nc.partition_id_tensor.name if nc.partition_id_tensor else None
    in_names, out_names, out_avals, zero_outs = [], [], [], []
    for alloc in nc.m.functions[0].allocations:
        if not isinstance(alloc, mb.MemoryLocationSet):
            continue
        name = alloc.memorylocations[0].name
        if alloc.kind == "ExternalInput":
            if name != pid_name:
                in_names.append(name)
        elif alloc.kind == "ExternalOutput":
            out_names.append(name)
            shape = tuple(alloc.tensor_shape)
            dtype = mb.dt.np(alloc.dtype)
            out_avals.append(jax.core.ShapedArray(shape, dtype))
            zero_outs.append(np.zeros(shape, dtype))
    n_params = len(in_names)
    n_outs = len(out_avals)
    all_names = in_names + out_names
    if pid_name is not None:
        all_names = all_names + [pid_name]

    def _body(*args):
        ins = list(args[:n_params])
        outs = list(args[n_params:])
        pid = [partition_id_tensor()] if pid_name is not None else []
        outs = list(_bass_exec_p.bind(
            *ins, *outs, *pid,
            out_avals=tuple(out_avals),
            in_names=tuple(all_names),
            out_names=tuple(out_names),
            lowering_input_output_aliases=(),
            sim_require_finite=True,
            sim_require_nnan=True,
            nc=nc,
        ))
        return tuple(outs)

    devices = jax.devices()[:NCORES]
    mesh = Mesh(np.asarray(devices), ("core",))
    in_specs = (PartitionSpec("core"),) * (n_params + n_outs)
    out_specs = (PartitionSpec("core"),) * n_outs
    # No donation: inputs and the zero "output seed" buffers stay resident on
    # device, so repeated calls measure dispatch+execute only.
    jfn = jax.jit(
        shard_map(_body, mesh=mesh, in_specs=in_specs,
                  out_specs=out_specs, check_rep=False),
        keep_unused=True,
    )

    from jax.sharding import NamedSharding
    sh = NamedSharding(mesh, PartitionSpec("core"))
    per_core = [[np.asarray(m[name]) for name in in_names] for m in in_maps]
    dev_in = [
        jax.device_put(
            np.concatenate([per_core[c][i] for c in range(NCORES)], axis=0), sh)
        for i in range(n_params)
    ]
    dev_zero = [
        jax.device_put(
            np.zeros((NCORES * z.shape[0], *z.shape[1:]), z.dtype), sh)
        for z in zero_outs
    ]

    out = jfn(*dev_in, *dev_zero)  # compile + warmup
    jax.block_until_ready(out)

    t1s, tms = [], []
    for _ in range(reps):
        t0 = _time.perf_counter()
        out = jfn(*dev_in, *dev_zero)
        jax.block_until_ready(out)
        t1s.append(_time.perf_counter() - t0)
    for _ in range(reps):
        t0 = _time.perf_counter()
        outs = [jfn(*dev_in, *dev_zero) for _ in range(iters)]
        jax.block_until_ready(outs)
        tms.append(_time.perf_counter() - t0)
    t1 = min(t1s)
    tm = min(tms)
    ns = (tm - t1) / (iters - 1) * 1e9
    return ns, {"t1_s": t1, "tm_s": tm, "iters": iters,
                "wall_ns_per_call": tm / iters * 1e9}


# revision 27
# speedup vs baseline: 1.1220x; 1.1220x over previous
"""Trainium2 Bass kernel for nn_EvoAttentionCausalTorch.

Reference math (per (b,h) slice, V: [L, D], D=128):
    ctx   = cumsum_l(V)
    cm    = ctx / t                      (t = 1..L)
    csg   = sigmoid(cm @ Wg + bg)
    s     = cumsum_l(csg * V)
    li    = V @ Wv + bv
    den   = |s| + |li| + 1e-8
    q     = V @ Wq + bq
    gate  = silu(q / den) * V
    fg    = sigmoid(gate @ Wf + bf)
    alive = (sum_d |V| > 0)
    y     = fg * ctx * alive
    out   = LN_d(y) * gamma + beta

Device mapping (per core, 8 of the 64 (b*h) slices):
  - "transposed" layout: d=128 on partitions, l on the free dim.
  - cumsums     -> DVE tensor_tensor_scan (prefix scan along free dim)
  - linears     -> PE matmul, lhsT = W (d_in x d_out), rhs = x^T chunks
  - sigmoid/abs/square -> ACT (single 'sigmoid_and_others' table set);
    silu(x) = x*sigmoid(x) with the multiply on DVE.
  - division    -> DVE reciprocal
  - LN stats    -> PE ones-matmul reductions (M=32 replicated col-tiles),
                   evicted via ACT, reshaped to [128, L/128] via SBUF DMA,
                   rsqrt = ACT Sqrt + DVE reciprocal + 1 Newton step.
  - LN apply    -> per-l rows A=rstd*alive, B=-mu*A broadcast via K=1
                   matmuls with lhsT=gamma row: E=gamma*A, Dp=gamma*B;
                   out = y*E + Dp + beta.
"""

import os
import sys
from contextlib import ExitStack

import numpy as np

for _p in ("/opt/trn_rl_repo", "/root/.axon_site/_ro/trn_rl_repo"):
    if os.path.isdir(_p) and _p not in sys.path:
        sys.path.insert(0, _p)

import ml_dtypes  # noqa: E402
import concourse.bass as bass  # noqa: E402
import concourse.mybir as mybir  # noqa: E402
import concourse.tile as tile  # noqa: E402

from concourse.vector_clock import ScopedClock  # noqa: E402

AF = mybir.ActivationFunctionType
OP = mybir.AluOpType
F32 = mybir.dt.float32
BF16 = mybir.dt.bfloat16

class SplitDrainTileContext(tile.TileContext):
    """TileContext that never leaves more than one semaphore wait on any
    instruction. The walrus build in this environment rejects instructions
    with multiple sync-wait commands ("Too many sync wait commands"), while
    the Tile scheduler freely emits them. Excess waits are hoisted onto
    same-engine NOP instructions inserted immediately before the owner."""

    def _split_multiwait_insts(self):
        nc = self.nc
        cur_bb = nc.cur_bb.bb
        for blk in nc.m.functions[0].blocks:
            insts = list(blk.instructions)
            if not any(
                i.sync_info is not None and len(i.sync_info.on_wait) > 1
                for i in insts
            ):
                continue
            out = []
            for inst in insts:
                si = inst.sync_info
                if si is not None and len(si.on_wait) > 1:
                    waits = list(si.on_wait)
                    eng = nc.engines[inst.engine]
                    for w in waits[:-1]:
                        nop = eng.nop(nofuse=True, hint="wait_split").ins
                        # nop() appended to the current bb; relocate it.
                        cl = cur_bb.instructions
                        assert cl[-1] is nop
                        cl.pop()
                        nop.sync_info = mybir.SyncInfo(
                            on_wait=[w], on_update=[])
                        out.append(nop)
                    inst.sync_info = mybir.SyncInfo(
                        on_wait=[waits[-1]], on_update=list(si.on_update))
                out.append(inst)
            blk.instructions.clear()
            blk.instructions.extend(out)

    def _drain_and_barrier(self, tick_clock, wait_clock):
        nc = self.nc
        self._split_multiwait_insts()
        drain_inst = nc.sync.drain()
        wait_clock.add_sem_waits(
            drain_inst.ins, ScopedClock({None: tick_clock.global_clock})
        )
        si = drain_inst.ins.sync_info
        if si is not None and len(si.on_wait) > 1:
            waits = list(si.on_wait)
            drain_inst.ins.sync_info = mybir.SyncInfo(
                on_wait=[waits[0]], on_update=list(si.on_update))
            for w in waits[1:]:
                d2 = nc.sync.drain()
                d2.ins.sync_info = mybir.SyncInfo(on_wait=[w], on_update=[])

        nc.all_engine_barrier()
        assert self.sems is not None
        popped = nc._tile_sem_poison_stack.pop()
        assert popped is self._sem_poison
        nc.clear_and_free_semaphores(list(self.sems.allocated().values()))
        nc.all_engine_barrier()


B, H, L, D = 4, 16, 4096, 128
NCORES = 8
S = (B * H) // NCORES  # slices per core
CH = 512               # chunk (free-dim) size; one PSUM bank of fp32
LN_EPS = 1e-5


def build_nc(S=S, L=L, CH=CH):
    NCH = L // CH
    Q = CH // 128  # sub-index within a chunk: l = CH*c + Q*p + q

    nc = bass.Bass(trn_type="TRN2")

    vt_d = nc.declare_dram_parameter("vt", [S, 128, L], F32, isOutput=False)
    wg_d = nc.declare_dram_parameter("wg", [128, 128], F32, isOutput=False)
    wv_d = nc.declare_dram_parameter("wv", [128, 128], F32, isOutput=False)
    wq_d = nc.declare_dram_parameter("wq", [128, 128], F32, isOutput=False)
    wf_d = nc.declare_dram_parameter("wf", [128, 128], F32, isOutput=False)
    bg_d = nc.declare_dram_parameter("bg", [128, 1], F32, isOutput=False)
    bv_d = nc.declare_dram_parameter("bv", [128, 1], F32, isOutput=False)
    bq_d = nc.declare_dram_parameter("bq", [128, 1], F32, isOutput=False)
    bf_d = nc.declare_dram_parameter("bf", [128, 1], F32, isOutput=False)
    grow_d = nc.declare_dram_parameter("grow", [1, 128], F32, isOutput=False)
    beta_d = nc.declare_dram_parameter("beta", [128, 1], F32, isOutput=False)
    invt_d = nc.declare_dram_parameter("invt", [128, L], F32, isOutput=False)
    out_d = nc.declare_dram_parameter("out_t", [S, 128, L], F32, isOutput=True)

    with SplitDrainTileContext(nc) as tc:
        with ExitStack() as ctx:
            const = ctx.enter_context(tc.tile_pool(name="const", bufs=1))
            big = ctx.enter_context(tc.tile_pool(name="big", bufs=2))
            mid = ctx.enter_context(tc.tile_pool(name="mid", bufs=2))
            sm = ctx.enter_context(tc.tile_pool(name="sm", bufs=2))
            pmm = ctx.enter_context(tc.tile_pool(name="pmm", bufs=1, space="PSUM"))
            pst = ctx.enter_context(tc.tile_pool(name="pst", bufs=2, space="PSUM"))
            ped = ctx.enter_context(tc.tile_pool(name="ped", bufs=1, space="PSUM"))

            # ---- constants ----
            def cload(name, shape, dt, dram):
                t = const.tile(shape, dt, tag=name)
                nc.sync.dma_start(out=t, in_=dram[:, :])
                return t

            wg = cload("wg", [128, 128], F32, wg_d)
            wv = cload("wv", [128, 128], F32, wv_d)
            wq = cload("wq", [128, 128], F32, wq_d)
            wf = cload("wf", [128, 128], F32, wf_d)
            bg_c = cload("bg", [128, 1], F32, bg_d)
            bv_c = cload("bv", [128, 1], F32, bv_d)
            bq_c = cload("bq", [128, 1], F32, bq_d)
            bf_c = cload("bf", [128, 1], F32, bf_d)
            grow = cload("grow", [1, 128], F32, grow_d)
            beta_c = cload("beta", [128, 1], F32, beta_d)
            invt = cload("invt", [128, L], F32, invt_d)
            ones_f = const.tile([128, 32], F32, tag="ones_f")
            nc.vector.memset(ones_f, 1.0)
            ones_b = const.tile([128, 32], BF16, tag="ones_b")
            nc.vector.memset(ones_b, 1.0)

            for s in range(S):
                vt = big.tile([128, L], F32, tag="vt")
                nc.sync.dma_start(out=vt, in_=vt_d[s, :, :])

                # ctx is kept as per-chunk LOCAL cumsums plus a per-partition
                # chunk offset (oc): true ctx[:, c*CH+j] = ctxt[:, c*CH+j] +
                # oc[c]. Bounding each scan to CH elements keeps fp32
                # accumulation noise ~sqrt(L/CH) lower; the offset add is
                # folded into downstream op slots for free.
                ctxt = big.tile([128, L], F32, tag="ctx")
                oc = [None] * (NCH + 1)
                oc0 = sm.tile([128, 1], F32, tag="oc0")
                nc.vector.memset(oc0, 0.0)
                oc[0] = oc0
                for c in range(NCH):
                    cs = slice(c * CH, (c + 1) * CH)
                    nc.vector.tensor_tensor_scan(
                        out=ctxt[:, cs], data0=vt[:, cs], data1=vt[:, cs],
                        initial=0.0, op0=OP.add, op1=OP.bypass)
                    if c + 1 <= NCH - 1:
                        oc_next = sm.tile([128, 1], F32, tag=f"oc{c + 1}")
                        nc.vector.tensor_scalar(
                            out=oc_next,
                            in0=ctxt[:, (c + 1) * CH - 1:(c + 1) * CH],
                            scalar1=oc[c], scalar2=None, op0=OP.add)
                        oc[c + 1] = oc_next

                y = big.tile([128, L], F32, tag="y")
                mu_t = sm.tile([128, NCH, Q], F32, tag="mu")
                sq_t = sm.tile([128, NCH, Q], F32, tag="sq")
                zz_t = sm.tile([128, NCH, Q], F32, tag="zz")

                os_ = [None] * (NCH + 1)
                os0 = sm.tile([128, 1], F32, tag="os0")
                nc.vector.memset(os0, 0.0)
                os_[0] = os0
                for c in range(NCH):
                    cs = slice(c * CH, (c + 1) * CH)

                    # causal-mean gate: csg = sigmoid(Wg@(ctx/t) + bg)
                    cm = mid.tile([128, CH], F32, tag="cm")
                    nc.vector.scalar_tensor_tensor(
                        out=cm, in0=ctxt[:, cs], scalar=oc[c],
                        in1=invt[:, cs], op0=OP.add, op1=OP.mult)
                    pg = pmm.tile([128, CH], F32, tag="mmg")
                    nc.tensor.matmul(out=pg, lhsT=wg, rhs=cm, start=True, stop=True)
                    t1 = mid.tile([128, CH], F32, tag="t1", bufs=3)
                    nc.scalar.activation(out=t1, in_=pg, func=AF.Sigmoid,
                                         bias=bg_c, scale=1.0)
                    sv = mid.tile([128, CH], F32, tag="sv", bufs=3)
                    nc.vector.tensor_mul(out=sv, in0=t1, in1=vt[:, cs])
                    # s = cumsum(csg*V): local scan + per-partition offset
                    s2 = mid.tile([128, CH], F32, tag="s2", bufs=3)
                    nc.vector.tensor_tensor_scan(
                        out=s2, data0=sv, data1=sv, initial=0.0,
                        op0=OP.add, op1=OP.bypass)
                    if c + 1 <= NCH - 1:
                        os_next = sm.tile([128, 1], F32, tag=f"os{c + 1}")
                        nc.vector.tensor_scalar(
                            out=os_next, in0=s2[:, CH - 1:CH],
                            scalar1=os_[c], scalar2=None, op0=OP.add)
                        os_[c + 1] = os_next

                    # local_info: |mmv + bv|
                    pv = pmm.tile([128, CH], F32, tag="mmv")
                    nc.tensor.matmul(out=pv, lhsT=wv, rhs=vt[:, cs],
                                     start=True, stop=True)
                    ali = mid.tile([128, CH], F32, tag="ali")
                    nc.scalar.activation(out=ali, in_=pv, func=AF.Abs,
                                         bias=bv_c, scale=1.0)
                    # den = (|s2 + os| + 1e-8) + |li| ; offset folds into Abs bias
                    as_ = mid.tile([128, CH], F32, tag="as")
                    nc.scalar.activation(out=as_, in_=s2, func=AF.Abs,
                                         bias=os_[c], scale=1.0)
                    den = mid.tile([128, CH], F32, tag="den", bufs=3)
                    nc.vector.scalar_tensor_tensor(
                        out=den, in0=as_, scalar=1e-8, in1=ali,
                        op0=OP.add, op1=OP.add)
                    rr = mid.tile([128, CH], F32, tag="rr")
                    nc.vector.reciprocal(out=rr, in_=den)

                    # qn = (mmq + bq)/den ; silu(qn) = qn*sigmoid(qn)
                    pq = pmm.tile([128, CH], F32, tag="mmq")
                    nc.tensor.matmul(out=pq, lhsT=wq, rhs=vt[:, cs],
                                     start=True, stop=True)
                    qh = mid.tile([128, CH], BF16, tag="qh", bufs=3)
                    nc.vector.scalar_tensor_tensor(
                        out=qh, in0=pq, scalar=bq_c, in1=rr,
                        op0=OP.add, op1=OP.mult)
                    s3 = mid.tile([128, CH], BF16, tag="s3")
                    nc.scalar.activation(out=s3, in_=qh, func=AF.Sigmoid,
                                         bias=0.0, scale=1.0)
                    sl_ = mid.tile([128, CH], BF16, tag="sl")
                    nc.vector.tensor_mul(out=sl_, in0=qh, in1=s3)
                    gate = mid.tile([128, CH], F32, tag="gate")
                    nc.vector.tensor_mul(out=gate, in0=sl_, in1=vt[:, cs])

                    # final gate: y = sigmoid(mmf + bf) * ctx
                    pf = pmm.tile([128, CH], F32, tag="mmf")
                    nc.tensor.matmul(out=pf, lhsT=wf, rhs=gate,
                                     start=True, stop=True)
                    t2 = mid.tile([128, CH], F32, tag="t2")
                    nc.scalar.activation(out=t2, in_=pf, func=AF.Sigmoid,
                                         bias=bf_c, scale=1.0)
                    # y = fg * (ctx_local + oc)
                    nc.vector.scalar_tensor_tensor(
                        out=y[:, cs], in0=ctxt[:, cs], scalar=oc[c],
                        in1=t2, op0=OP.add, op1=OP.mult)

                    # LN stats + alive rows via PE reductions
                    y2 = mid.tile([128, CH], F32, tag="y2")
                    nc.scalar.activation(out=y2, in_=y[:, cs], func=AF.Square)
                    av = mid.tile([128, CH], BF16, tag="av")
                    nc.scalar.activation(out=av, in_=vt[:, cs], func=AF.Abs)

                    pstat = pst.tile([96, CH], F32, tag="st")
                    nc.tensor.matmul(out=pstat[0:32, :], lhsT=ones_f,
                                     rhs=y[:, cs], start=True, stop=True,
                                     tile_position=(0, 0))
                    nc.tensor.matmul(out=pstat[32:64, :], lhsT=ones_f,
                                     rhs=y2, start=True, stop=True,
                                     tile_position=(0, 32))
                    nc.tensor.matmul(out=pstat[64:96, :], lhsT=ones_b,
                                     rhs=av, start=True, stop=True,
                                     tile_position=(0, 64))
                    strows = mid.tile([96, CH], F32, tag="strows")
                    nc.scalar.activation(out=strows, in_=pstat, func=AF.Copy)
                    # reshape rows [1, CH] -> [128, Q] ; l = CH*c + Q*p + q
                    nc.sync.dma_start(out=mu_t[:, c, :], in_=strows[0:1, :])
                    nc.sync.dma_start(out=sq_t[:, c, :], in_=strows[32:33, :])
                    nc.sync.dma_start(out=zz_t[:, c, :], in_=strows[64:65, :])

                # ---- per-slice LN row math on [128, NCH*Q] ----
                mu_n = sm.tile([128, NCH, Q], F32, tag="mun")
                nc.vector.tensor_scalar(out=mu_n, in0=mu_t, scalar1=1.0 / 128,
                                        scalar2=None, op0=OP.mult)
                m2 = sm.tile([128, NCH, Q], F32, tag="m2")
                nc.vector.tensor_mul(out=m2, in0=mu_n, in1=mu_n)
                w2 = sm.tile([128, NCH, Q], F32, tag="w2")
                nc.vector.scalar_tensor_tensor(
                    out=w2, in0=sq_t, scalar=1.0 / 128, in1=m2,
                    op0=OP.mult, op1=OP.subtract)
                nc.vector.tensor_scalar(out=w2, in0=w2, scalar1=LN_EPS,
                                        scalar2=None, op0=OP.add)
                std0 = sm.tile([128, NCH, Q], F32, tag="std0")
                nc.scalar.activation(out=std0, in_=w2, func=AF.Sqrt)
                rs0 = sm.tile([128, NCH, Q], F32, tag="rs0")
                nc.vector.reciprocal(out=rs0, in_=std0)
                # one Newton refinement: rs1 = rs0*(1.5 - 0.5*w2*rs0^2)
                tn = sm.tile([128, NCH, Q], F32, tag="tn")
                nc.vector.tensor_mul(out=tn, in0=w2, in1=rs0)
                tn2 = sm.tile([128, NCH, Q], F32, tag="tn2")
                nc.vector.tensor_mul(out=tn2, in0=tn, in1=rs0)
                un = sm.tile([128, NCH, Q], F32, tag="un")
                nc.vector.tensor_scalar(out=un, in0=tn2, scalar1=-0.5,
                                        scalar2=1.5, op0=OP.mult, op1=OP.add)
                rs1 = sm.tile([128, NCH, Q], F32, tag="rs1")
                nc.vector.tensor_mul(out=rs1, in0=rs0, in1=un)
                # alive mask and final per-l factors
                af = sm.tile([128, NCH, Q], F32, tag="af")
                nc.vector.tensor_scalar(out=af, in0=zz_t, scalar1=0.0,
                                        scalar2=None, op0=OP.is_gt)
                A_t = sm.tile([128, NCH, Q], F32, tag="A")
                nc.vector.tensor_mul(out=A_t, in0=rs1, in1=af)
                B_t = sm.tile([128, NCH, Q], F32, tag="Bt")
                nc.vector.scalar_tensor_tensor(
                    out=B_t, in0=mu_n, scalar=-1.0, in1=A_t,
                    op0=OP.mult, op1=OP.mult)

                # ---- apply: out = y*(gamma.A) + gamma.B + beta ----
                for c in range(NCH):
                    cs = slice(c * CH, (c + 1) * CH)
                    ab = mid.tile([1, 2 * CH], F32, tag="ab")
                    nc.sync.dma_start(out=ab[0:1, 0:CH], in_=A_t[:, c, :])
                    nc.sync.dma_start(out=ab[0:1, CH:2 * CH], in_=B_t[:, c, :])
                    pe = ped.tile([128, 2 * CH], F32, tag="ed")
                    nc.tensor.matmul(out=pe[:, 0:CH], lhsT=grow,
                                     rhs=ab[0:1, 0:CH], start=True, stop=True)
                    nc.tensor.matmul(out=pe[:, CH:2 * CH], lhsT=grow,
                                     rhs=ab[0:1, CH:2 * CH], start=True, stop=True)
                    zc = mid.tile([128, CH], F32, tag="zc")
                    nc.vector.tensor_mul(out=zc, in0=y[:, cs], in1=pe[:, 0:CH])
                    ou = mid.tile([128, CH], F32, tag="ou")
                    nc.vector.scalar_tensor_tensor(
                        out=ou, in0=zc, scalar=beta_c, in1=pe[:, CH:2 * CH],
                        op0=OP.add, op1=OP.add)
                    nc.sync.dma_start(out=out_d[s, :, cs], in_=ou)
    return nc


def host_consts(Wg, bg, Wv, bv, Wq, bq, Wf, bf, gamma, beta, L=L):
    t = np.arange(1, L + 1, dtype=np.float64)
    invt = np.broadcast_to((1.0 / t).astype(np.float32), (128, L))
    return {
        "wg": np.ascontiguousarray(Wg, dtype=np.float32),
        "wv": np.ascontiguousarray(Wv, dtype=np.float32),
        "wq": np.ascontiguousarray(Wq, dtype=np.float32),
        "wf": np.ascontiguousarray(Wf, dtype=np.float32),
        "bg": np.asarray(bg, dtype=np.float32).reshape(128, 1),
        "bv": np.asarray(bv, dtype=np.float32).reshape(128, 1),
        "bq": np.asarray(bq, dtype=np.float32).reshape(128, 1),
        "bf": np.asarray(bf, dtype=np.float32).reshape(128, 1),
        "grow": np.asarray(gamma, dtype=np.float32).reshape(1, 128),
        "beta": np.asarray(beta, dtype=np.float32).reshape(128, 1),
        "invt": np.ascontiguousarray(invt),
    }


_NC_CACHE = {}


def _get_nc():
    key = (S, L, CH)
    if key not in _NC_CACHE:
        _NC_CACHE[key] = build_nc(*key)
    return _NC_CACHE[key]


def run_kernel(inputs, trace=False):
    """Returns (output [B,H,L,D] fp32, exec_time_ns or None)."""
    from concourse.bass_utils import run_bass_kernel_spmd

    V = np.asarray(inputs["V"], dtype=np.float32)
    consts = host_consts(
        np.asarray(inputs["Wg"]), np.asarray(inputs["bg"]),
        np.asarray(inputs["Wv"]), np.asarray(inputs["bv"]),
        np.asarray(inputs["Wq"]), np.asarray(inputs["bq"]),
        np.asarray(inputs["Wf"]), np.asarray(inputs["bf"]),
        np.asarray(inputs["gamma"]), np.asarray(inputs["beta"]),
    )
    Vr = V.reshape(B * H, L, D)
    in_maps = []
    for c in range(NCORES):
        sl = Vr[c * S:(c + 1) * S]                       # [S, L, D]
        vt = np.ascontiguousarray(sl.transpose(0, 2, 1))  # [S, D, L]
        m = {"vt": vt}
        m.update(consts)
        in_maps.append(m)

    nc = _get_nc()
    res = run_bass_kernel_spmd(nc, in_maps, list(range(NCORES)), trace=trace)
    outs = [res.results[c]["out_t"] for c in range(NCORES)]
    out = np.concatenate(outs, axis=0)                   # [B*H, D, L]
    out = out.transpose(0, 2, 1).reshape(B, H, L, D)
    return np.ascontiguousarray(out, dtype=np.float32), res.exec_time_ns


def kernel(**inputs):
    out, _ = run_kernel(inputs, trace=False)
    return out


def _in_maps_from_inputs(inputs):
    V = np.asarray(inputs["V"], dtype=np.float32)
    consts = host_consts(
        np.asarray(inputs["Wg"]), np.asarray(inputs["bg"]),
        np.asarray(inputs["Wv"]), np.asarray(inputs["bv"]),
        np.asarray(inputs["Wq"]), np.asarray(inputs["bq"]),
        np.asarray(inputs["Wf"]), np.asarray(inputs["bf"]),
        np.asarray(inputs["gamma"]), np.asarray(inputs["beta"]),
    )
    Vr = V.reshape(B * H, L, D)
    in_maps = []
    for c in range(NCORES):
        sl = Vr[c * S:(c + 1) * S]
        vt = np.ascontiguousarray(sl.transpose(0, 2, 1))
        m = {"vt": vt}
        m.update(consts)
        in_maps.append(m)
    return in_maps


def time_kernel(inputs, iters=12, reps=3):
    """Estimate per-invocation NEFF execution time by chaining `iters`
    back-to-back bass_exec calls inside one jitted program (the outputs of
    call i feed the donated output buffers of call i+1, forcing sequential
    execution and defeating CSE). Returns (ns_per_iter, details)."""
    import jax
    from jax.experimental.shard_map import shard_map
    from jax.sharding import Mesh, PartitionSpec
    import time as _time

    from concourse import bass2jax, mybir as mb
    from concourse.bass2jax import (
        _bass_exec_p, install_neuronx_cc_hook, partition_id_tensor,
    )

    install_neuronx_cc_hook()
    nc = _get_nc()
    in_maps = _in_maps_from_inputs(inputs)

    pid_name = nc.partition_id_tensor.name if nc.partition_id_tensor else None
    in_names, out_names, out_avals, zero_outs = [], [], [], []
    for alloc in nc.m.functions[0].allocations:
        if not isinstance(alloc, mb.MemoryLocationSet):
            continue
        name = alloc.memorylocations[0].name
        if alloc.kind == "ExternalInput":
            if name != pid_name:
                in_names.append(name)
        elif alloc.kind == "ExternalOutput":
            out_names.append(name)
            shape = tuple(alloc.tensor_shape)
            dtype = mb.dt.np(alloc.dtype)
            out_avals.append(jax.core.ShapedArray(shape, dtype))
            zero_outs.append(np.zeros(shape, dtype))
    n_params = len(in_names)
    n_outs = len(out_avals)
    all_names = in_names + out_names
    if pid_name is not None:
        all_names = all_names + [pid_name]

    def _body(*args):
        ins = list(args[:n_params])
        outs = list(args[n_params:])
        pid = [partition_id_tensor()] if pid_name is not None else []
        outs = list(_bass_exec_p.bind(
            *ins, *outs, *pid,
            out_avals=tuple(out_avals),
            in_names=tuple(all_names),
            out_names=tuple(out_names),
            lowering_input_output_aliases=(),
            sim_require_finite=True,
            sim_require_nnan=True,
            nc=nc,
        ))
        return tuple(outs)

    devices = jax.devices()[:NCORES]
    mesh = Mesh(np.asarray(devices), ("core",))
    in_specs = (PartitionSpec("core"),) * (n_params + n_outs)
    out_specs = (PartitionSpec("core"),) * n_outs
    # No donation: inputs and the zero "output seed" buffers stay resident on
    # device, so repeated calls measure dispatch+execute only.
    jfn = jax.jit(
        shard_map(_body, mesh=mesh, in_specs=in_specs,
                  out_specs=out_specs, check_rep=False),
        keep_unused=True,
    )

    from jax.sharding import NamedSharding
    sh = NamedSharding(mesh, PartitionSpec("core"))
    per_core = [[np.asarray(m[name]) for name in in_names] for m in in_maps]
    dev_in = [
        jax.device_put(
            np.concatenate([per_core[c][i] for c in range(NCORES)], axis=0), sh)
        for i in range(n_params)
    ]
    dev_zero = [
        jax.device_put(
            np.zeros((NCORES * z.shape[0], *z.shape[1:]), z.dtype), sh)
        for z in zero_outs
    ]

    out = jfn(*dev_in, *dev_zero)  # compile + warmup
    jax.block_until_ready(out)

    t1s, tms = [], []
    for _ in range(reps):
        t0 = _time.perf_counter()
        out = jfn(*dev_in, *dev_zero)
        jax.block_until_ready(out)
        t1s.append(_time.perf_counter() - t0)
    for _ in range(reps):
        t0 = _time.perf_counter()
        outs = [jfn(*dev_in, *dev_zero) for _ in range(iters)]
        jax.block_until_ready(outs)
        tms.append(_time.perf_counter() - t0)
    t1 = min(t1s)
    tm = min(tms)
    ns = (tm - t1) / (iters - 1) * 1e9
    base = _dispatch_baseline_ns(iters, reps)
    corrected = max(0.0, ns - base) if base is not None else ns
    return corrected, {
        "t1_s": t1, "tm_s": tm, "iters": iters,
        "marginal_ns_per_call": ns,
        "dispatch_baseline_ns": base,
        "wall_ns_per_call": tm / iters * 1e9,
    }


def _dispatch_baseline_ns(iters, reps):
    """Marginal per-call time of a near-empty kernel: the axon/PJRT dispatch
    floor, subtracted from the full kernel's marginal time."""
    import jax
    import time as _time
    from jax.experimental.shard_map import shard_map
    from jax.sharding import Mesh, NamedSharding, PartitionSpec

    from concourse.bass2jax import (
        _bass_exec_p, install_neuronx_cc_hook, partition_id_tensor,
    )

    try:
        install_neuronx_cc_hook()
        nc = bass.Bass(trn_type="TRN2")
        x_d = nc.declare_dram_parameter("x", [128, 128], F32, isOutput=False)
        y_d = nc.declare_dram_parameter("y", [128, 128], F32, isOutput=True)
        with SplitDrainTileContext(nc) as tc:
            with ExitStack() as ctx:
                pool = ctx.enter_context(tc.tile_pool(name="p", bufs=2))
                t = pool.tile([128, 128], F32)
                nc.sync.dma_start(out=t, in_=x_d[:, :])
                t2 = pool.tile([128, 128], F32)
                nc.vector.tensor_scalar(out=t2, in0=t, scalar1=2.0,
                                        scalar2=None, op0=OP.mult)
                nc.sync.dma_start(out=y_d[:, :], in_=t2)

        pid_name = (nc.partition_id_tensor.name
                    if nc.partition_id_tensor else None)
        names = ["x", "y"] + ([pid_name] if pid_name else [])

        def _body(x, yz):
            pid = [partition_id_tensor()] if pid_name else []
            import jax.core as jcore
            outs = _bass_exec_p.bind(
                x, yz, *pid,
                out_avals=(jcore.ShapedArray((128, 128), np.float32),),
                in_names=tuple(names), out_names=("y",),
                lowering_input_output_aliases=(),
                sim_require_finite=True, sim_require_nnan=True, nc=nc)
            return tuple(outs)

        mesh = Mesh(np.asarray(jax.devices()[:NCORES]), ("core",))
        sh = NamedSharding(mesh, PartitionSpec("core"))
        jfn = jax.jit(
            shard_map(_body, mesh=mesh,
                      in_specs=(PartitionSpec("core"),) * 2,
                      out_specs=(PartitionSpec("core"),), check_rep=False),
            keep_unused=True)
        X = jax.device_put(
            np.zeros((NCORES * 128, 128), np.float32), sh)
        Z = jax.device_put(
            np.zeros((NCORES * 128, 128), np.float32), sh)
        out = jfn(X, Z)
        jax.block_until_ready(out)
        t1s, tms = [], []
        for _ in range(reps):
            t0 = _time.perf_counter()
            out = jfn(X, Z)
            jax.block_until_ready(out)
            t1s.append(_time.perf_counter() - t0)
        for _ in range(reps):
            t0 = _time.perf_counter()
            outs = [jfn(X, Z) for _ in range(iters)]
            jax.block_until_ready(outs)
            tms.append(_time.perf_counter() - t0)
        return (min(tms) - min(t1s)) / (iters - 1) * 1e9
    except Exception:
        return None
